# revision 4
# baseline (speedup 1.0000x reference)
"""Distributed embedding-lookup kernel for 8 Trainium2 NeuronCores.

Reference computation (B=16384, D=128, CTX=8, S=10):
    inputs = paragraph_matrix[doc_ids] + sum(word_matrix[context_ids], axis=1)
    logits = einsum("bd,dbs->bs", inputs, outputs[:, sample_ids])

Strategy: data-parallel over the batch. Each core processes B/8 = 2048 rows;
the three tables are replicated. All row lookups are 512-byte indirect DMA
gathers (one offset per destination partition, 128 rows per instruction),
context vectors are tree-summed on the vector engine, and the sample dot
products are an elementwise multiply + free-axis reduction.

kernel(**inputs) takes the full unsharded inputs and returns the full
[16384, 10] float32 logits.
"""
import sys

if '/opt/trn_rl_repo' not in sys.path:
    sys.path.insert(0, '/opt/trn_rl_repo')

import numpy as np

N_DOCS = 1_000_000
N_WORDS = 100_000
BATCH = 16384
N_CORES = 8
B_CORE = BATCH // N_CORES   # 2048
CTX = 8
S = 10
D = 128
P = 128
BT = B_CORE // P            # 16 btiles per core

_CACHE = {}


def _build_nc(t_chunk=1):
    import concourse.bass as bass
    import concourse.mybir as mybir
    import concourse.tile as tile
    from concourse import bacc

    assert BT % t_chunk == 0
    nchunk = BT // t_chunk
    T = t_chunk

    nc = bacc.Bacc("TRN2", target_bir_lowering=False, debug=False)
    par = nc.dram_tensor("par", [N_DOCS, D], mybir.dt.float32, kind="ExternalInput")
    wrd = nc.dram_tensor("wrd", [N_WORDS, D], mybir.dt.float32, kind="ExternalInput")
    outT = nc.dram_tensor("outT", [N_WORDS, D], mybir.dt.float32, kind="ExternalInput")
    doc_idx = nc.dram_tensor("doc_idx", [P, BT], mybir.dt.int32, kind="ExternalInput")
    ctx_idx = nc.dram_tensor("ctx_idx", [P, BT * CTX], mybir.dt.int32, kind="ExternalInput")
    smp_idx = nc.dram_tensor("smp_idx", [P, BT * S], mybir.dt.int32, kind="ExternalInput")
    logits = nc.dram_tensor("logits", [B_CORE, S], mybir.dt.float32, kind="ExternalOutput")

    with tile.TileContext(nc) as tc:
        with (
            tc.tile_pool(name="idx", bufs=1) as idx_pool,
            tc.tile_pool(name="par", bufs=12) as par_pool,
            tc.tile_pool(name="ctx", bufs=12) as ctx_pool,
            tc.tile_pool(name="smp", bufs=12) as smp_pool,
            tc.tile_pool(name="lg", bufs=12) as lg_pool,
        ):
            doc_sb = idx_pool.tile([P, BT], mybir.dt.int32, tag="doc")
            ctx_sb = idx_pool.tile([P, BT * CTX], mybir.dt.int32, tag="ctx")
            smp_sb = idx_pool.tile([P, BT * S], mybir.dt.int32, tag="smp")
            nc.sync.dma_start(doc_sb[:], doc_idx.ap())
            nc.sync.dma_start(ctx_sb[:], ctx_idx.ap())
            nc.sync.dma_start(smp_sb[:], smp_idx.ap())

            lg_dram = logits.ap()

            for t in range(nchunk):
                par_t = par_pool.tile([P, T * D], mybir.dt.float32, tag="par")
                ctx_t = ctx_pool.tile([P, T * CTX * D], mybir.dt.float32, tag="ctx")
                smp_t = smp_pool.tile([P, T * S * D], mybir.dt.float32, tag="smp")

                # One offset per dest partition -> 128 rows per indirect DMA.
                for j in range(T):
                    nc.gpsimd.indirect_dma_start(
                        out=par_t[:, j * D:(j + 1) * D], out_offset=None, in_=par.ap(),
                        in_offset=bass.IndirectOffsetOnAxis(
                            ap=doc_sb[:, t * T + j:t * T + j + 1], axis=0),
                    )
                    for u in range(CTX):
                        m = j * CTX + u
                        col = (t * T + j) * CTX + u
                        nc.gpsimd.indirect_dma_start(
                            out=ctx_t[:, m * D:(m + 1) * D], out_offset=None, in_=wrd.ap(),
                            in_offset=bass.IndirectOffsetOnAxis(
                                ap=ctx_sb[:, col:col + 1], axis=0),
                        )
                    for s in range(S):
                        m = j * S + s
                        col = (t * T + j) * S + s
                        nc.gpsimd.indirect_dma_start(
                            out=smp_t[:, m * D:(m + 1) * D], out_offset=None, in_=outT.ap(),
                            in_offset=bass.IndirectOffsetOnAxis(
                                ap=smp_sb[:, col:col + 1], axis=0),
                        )

                ctx4 = ctx_t[:].rearrange("p (j u d) -> p j u d", u=CTX, d=D)
                nc.vector.tensor_add(ctx4[:, :, 0:4, :], ctx4[:, :, 0:4, :], ctx4[:, :, 4:8, :])
                nc.vector.tensor_add(ctx4[:, :, 0:2, :], ctx4[:, :, 0:2, :], ctx4[:, :, 2:4, :])
                nc.vector.tensor_add(ctx4[:, :, 0:1, :], ctx4[:, :, 0:1, :], ctx4[:, :, 1:2, :])

                par3 = par_t[:].rearrange("p (j d) -> p j d", d=D)
                nc.vector.tensor_add(par3, par3, ctx4[:, :, 0, :])

                smp4 = smp_t[:].rearrange("p (j s d) -> p j s d", s=S, d=D)
                par_bc = bass.AP(par3.tensor, par3.offset,
                                 [par3.ap[0], par3.ap[1], [0, S], par3.ap[2]])
                nc.vector.tensor_mul(smp4, smp4, par_bc)

                lg_t = lg_pool.tile([P, T * S], mybir.dt.float32, tag="lg")
                nc.vector.reduce_sum(
                    lg_t[:], smp_t[:].rearrange("p (m d) -> p m d", d=D),
                    axis=mybir.AxisListType.X,
                )

                dram_rows = lg_dram[t * T * P:(t + 1) * T * P, :]
                dram_v = dram_rows.rearrange("(j p) s -> p j s", p=P)
                sb_v = lg_t[:].rearrange("p (j s) -> p j s", s=S)
                nc.sync.dma_start(dram_v, sb_v)
    nc.compile()
    return nc


def _get_nc():
    if "nc" not in _CACHE:
        _CACHE["nc"] = _build_nc()
    return _CACHE["nc"]


def kernel(doc_ids, context_ids, sample_ids, paragraph_matrix, word_matrix, outputs):
    from concourse import bass_utils

    doc_ids = np.asarray(doc_ids).astype(np.int32)
    context_ids = np.asarray(context_ids).astype(np.int32)
    sample_ids = np.asarray(sample_ids).astype(np.int32)
    par = np.ascontiguousarray(np.asarray(paragraph_matrix), dtype=np.float32)
    wrd = np.ascontiguousarray(np.asarray(word_matrix), dtype=np.float32)
    outT = np.ascontiguousarray(np.asarray(outputs, dtype=np.float32).T)

    nc = _get_nc()

    in_maps = []
    for c in range(N_CORES):
        sl = slice(c * B_CORE, (c + 1) * B_CORE)
        d = doc_ids[sl].reshape(BT, P).T.copy()
        cx = (context_ids[sl].reshape(BT, P, CTX)
              .transpose(1, 0, 2).reshape(P, BT * CTX).copy())
        sp = (sample_ids[sl].reshape(BT, P, S)
              .transpose(1, 0, 2).reshape(P, BT * S).copy())
        in_maps.append({
            "par": par, "wrd": wrd, "outT": outT,
            "doc_idx": d, "ctx_idx": cx, "smp_idx": sp,
        })

    res = bass_utils.run_bass_kernel_spmd(
        nc, in_maps, core_ids=list(range(N_CORES)), trace=False)
    logits = np.concatenate(
        [res.results[c]["logits"] for c in range(N_CORES)], axis=0)
    return logits.astype(np.float32)
